# revision 3
# baseline (speedup 1.0000x reference)
"""Multi-head attention (B=4, S=2048, E=1024, H=16, causal) on 8 Trainium2 cores.

Sharding: core = (batch b, head-group g) — 4 batches x 2 groups of 8 heads.

v4: single fine-grained software pipeline.  The attention stream is a flat
list of ip-steps (unit-major, block-major); scores are emitted ONE step
ahead of the exp window (queue order per cycle: V(k-1), S(k+1), fillers)
so ScalarE exp runs back-to-back while the PE executes attn@V of the
previous step, scores of the next, plus projection/output-projection
matmul chunks pulled from a filler queue.  kv(j)/q(j) availability is
enforced by require() seams that drain the filler queue densely at the
first step that needs them.  Body+diag of each unit share one PSUM
accumulation (no ctx re-adds); row-sums ride the ones-columns of v_aug
and are normalized per half-pair with reciprocal_approx_fast.
DMA triggers never ride the Scalar queue (ACT is the pacer).
"""

import os
import sys

for _p in ("/opt/trn_rl_repo", "/root/.axon_site/_ro/trn_rl_repo"):
    if os.path.isdir(_p) and _p not in sys.path:
        sys.path.append(_p)

from collections import deque

import numpy as np
import ml_dtypes

import concourse.bacc as bacc
import concourse.mybir as mybir
from concourse import tile
from concourse import bass_utils
from concourse.bass import ts

BF16 = ml_dtypes.bfloat16
F32 = mybir.dt.float32
BF = mybir.dt.bfloat16
AFT = mybir.ActivationFunctionType
ALU = mybir.AluOpType

B, S, E = 4, 2048, 1024
H, D = 16, 64
G = 512            # head dims per core (8 heads)
KC = E // 128      # contraction chunks for projections
NM = G // 128      # m-tiles of the group dim
NJ = S // 512      # 512-wide token column blocks
NT = S // 128      # 128-wide token tiles

_NC = None


def _build():
    nc = bacc.Bacc("TRN2", target_bir_lowering=False, debug=False, num_devices=8)

    xq = nc.dram_tensor("xq", (E, S), BF, kind="ExternalInput").ap()
    xk = nc.dram_tensor("xk", (E, S), BF, kind="ExternalInput").ap()
    xv = nc.dram_tensor("xv", (E, S), BF, kind="ExternalInput").ap()
    wq = nc.dram_tensor("wq", (E, G), BF, kind="ExternalInput").ap()
    wk = nc.dram_tensor("wk", (E, G), BF, kind="ExternalInput").ap()
    wv = nc.dram_tensor("wv", (E, G), BF, kind="ExternalInput").ap()
    wo = nc.dram_tensor("wo", (G, E), BF, kind="ExternalInput").ap()
    qb = nc.dram_tensor("qb", (128, NM), F32, kind="ExternalInput").ap()
    kb = nc.dram_tensor("kb", (128, NM), F32, kind="ExternalInput").ap()
    sel = nc.dram_tensor("sel", (4, G), BF, kind="ExternalInput").ap()
    fT = nc.dram_tensor("fT", (E, S), BF, kind="ExternalOutput").ap()

    with tile.TileContext(nc) as tc:
        with (
            tc.tile_pool(name="cst", bufs=2) as cst,
            tc.tile_pool(name="wsb", bufs=24) as wsb,
            tc.tile_pool(name="xs", bufs=6) as xsp,
            tc.tile_pool(name="qt", bufs=8) as qtp,
            tc.tile_pool(name="va", bufs=16) as vap,
            tc.tile_pool(name="ctx", bufs=4) as ctxp,
            tc.tile_pool(name="exp", bufs=10) as expp,
            tc.tile_pool(name="wo", bufs=4) as wop,
            tc.tile_pool(name="fin", bufs=4) as finp,
            tc.tile_pool(name="rb", bufs=9) as rbp,
            tc.tile_pool(name="tmp", bufs=4) as tmpp,
            tc.tile_pool(name="psc", bufs=2, space="PSUM") as pscorep,
            tc.tile_pool(name="pcx", bufs=2, space="PSUM") as pctxp,
            tc.tile_pool(name="ppj", bufs=2, space="PSUM") as pprojp,
        ):
            qb_t = cst.tile([128, NM], F32, tag="cst")
            kb_t = cst.tile([128, NM], F32, tag="cst")
            sel_sb = cst.tile([4, G], BF, tag="sel", name="sel_sb")

            zero_fill = nc.gpsimd.to_reg(0.0)

            # Warm the ScalarE Exp table long before the first real exp.
            warm = cst.tile([1, 8], F32, tag="warm", name="warm")
            nc.vector.memset(warm[:, :], 0.0)
            nc.scalar.activation(warm[:, :], warm[:, :], AFT.Exp)

            # PE HAM warm-up on a zeroed tile while the initial DMAs land.
            wmt = cst.tile([128, 384], BF, tag="wmt", name="wmt")
            nc.vector.memset(wmt[:, :], 0.0)
            for wi in range(16):
                wps = pprojp.tile([128, 512], F32, tag="ppj", name=f"wps{wi}")
                nc.tensor.matmul(
                    wps[:, 0:256], wmt[:, 0:128], wmt[:, 128:384],
                    start=True, stop=True)
            nc.scalar.activation(warm[:, :], warm[:, :], AFT.Exp)

            # constant ones-slots pattern for v_aug cols [64..68) per head
            ones_c = cst.tile([128, 8 * 68], BF, tag="ones", name="ones_c")
            ones_c3 = ones_c[:, :].rearrange("p (h x) -> p h x", h=8)
            nc.vector.memset(ones_c3[:, :, 64:68], 0.0)
            for h in range(8):
                nc.vector.memset(
                    ones_c3[:, h : h + 1, 64 + (h % 4) : 65 + (h % 4)], 1.0)

            qT = [qtp.tile([128, S], BF, tag="qt", name=f"qT{m}") for m in range(NM)]
            kT = [qtp.tile([128, S], BF, tag="qt", name=f"kT{m}") for m in range(NM)]
            ctxT = [ctxp.tile([128, S], BF, tag="ctx", name=f"ctxT{m}")
                    for m in range(NM)]
            v_aug = [vap.tile([128, 8 * 68], BF, tag="va", name=f"va{t}")
                     for t in range(NT)]

            nc.gpsimd.dma_start(qb_t[:, :], qb[:, :])
            nc.gpsimd.dma_start(kb_t[:, :], kb[:, :])
            nc.gpsimd.dma_start(sel_sb[0:4, :], sel[:, :])

            # q/k/v weights resident for the whole kernel
            wq_sb = [wsb.tile([128, G], BF, tag="w", name=f"wq{kc}") for kc in range(KC)]
            wk_sb = [wsb.tile([128, G], BF, tag="w", name=f"wk{kc}") for kc in range(KC)]
            wv_sb = [wsb.tile([128, G], BF, tag="w", name=f"wv{kc}") for kc in range(KC)]
            for kc in range(KC):
                nc.gpsimd.dma_start(wq_sb[kc][:, :], wq[ts(kc, 128), :])
            for kc in range(KC):
                nc.sync.dma_start(wk_sb[kc][:, :], wk[ts(kc, 128), :])
            for kc in range(KC):
                nc.gpsimd.dma_start(wv_sb[kc][:, :], wv[ts(kc, 128), :])
            wo_sb = [wop.tile([128, E], BF, tag="wo", name=f"wo{ec}") for ec in range(NM)]
            for t in range(NT):
                va3i = v_aug[t][:, :].rearrange("p (h x) -> p h x", h=8)
                nc.gpsimd.tensor_copy(va3i[:, :, 64:68], ones_c3[:, :, 64:68])

            # ---- x-block staging (fine first block, 2 triggers later) ----
            _x_ap = {"q": xq, "k": xk, "v": xv}
            _xsb = {}

            def load_x(which, n):
                key = (which, n)
                if key in _xsb:
                    return _xsb[key]
                xsb = xsp.tile([128, KC * 512], BF, tag="xs", name=f"x{which}{n}")
                xs3 = xsb[:, :].rearrange("p (k c) -> p k c", k=KC)
                ap = _x_ap[which]
                if n == 0:
                    for kc in range(KC):
                        nc.sync.dma_start(xs3[:, kc, :], ap[ts(kc, 128), ts(n, 512)])
                else:
                    src = ap[:, ts(n, 512)].rearrange("(k p) c -> p k c", p=128)
                    nc.sync.dma_start(xs3[:, 0:4, :], src[:, 0:4, :])
                    nc.sync.dma_start(xs3[:, 4:8, :], src[:, 4:8, :])
                _xsb[key] = xs3
                return xs3

            # ---- filler queue --------------------------------------------
            # entries: (tag, n_mms, fn).  pump(b) emits entries until b MM
            # slots are consumed; require(tag) drains densely until every
            # entry of `tag` has been emitted.
            filler_q = deque()
            _remaining = {}
            done_tags = set()

            def _push(tag, mms, fn, is_start=False, front=False):
                _remaining[tag] = _remaining.get(tag, 0) + 1
                if front:
                    filler_q.appendleft((tag, mms, fn, is_start))
                else:
                    filler_q.append((tag, mms, fn, is_start))

            def _run_one():
                tag, mms, fn, _st = filler_q.popleft()
                fn()
                _remaining[tag] -= 1
                if _remaining[tag] == 0:
                    done_tags.add(tag)
                return max(mms, 1)

            _dummy_n = [0]

            def pump(budget):
                while budget > 0 and filler_q:
                    budget -= _run_one()
                if budget >= 2:
                    # queue dry: keep PE duty high so the HAM clock gate
                    # stays at 8/8 (dummies are ~107ns N=128 matmuls)
                    _dummy_n[0] += 1
                    wp = pprojp.tile([128, 512], F32, tag="ppj",
                                     name=f"dum{_dummy_n[0]}")
                    nc.tensor.matmul(wp[:, 0:128], wmt[:, 0:128],
                                     wmt[:, 128:256], start=True, stop=True)

            def require(tag):
                if tag in done_tags or tag not in _remaining:
                    return
                while tag not in done_tags:
                    _run_one()

            def drain_to_boundary():
                # Flush the remainder of any half-emitted chunk so that no
                # inline PSUM allocation (psn) can slot-wait on an epilogue
                # that would otherwise be emitted BEHIND it (PE-queue cycle).
                while filler_q and not filler_q[0][3]:
                    _run_one()

            # ---- projection chunk generators -----------------------------
            def push_qk(tag, n, which, w_sb, dst, bias_t, scale, prefetch):
                for mp in range(2):
                    subtag = tag + ("a" if mp == 0 else "b")
                    ps_box = {}

                    def _mk_step(kc, mp=mp, n=n, which=which, w_sb=w_sb,
                                 ps_box=ps_box, prefetch=prefetch):
                        def _step():
                            xs3 = load_x(which, n)
                            if kc == 0 and mp == 0 and prefetch is not None:
                                load_x(*prefetch)
                            if kc == 0:
                                ps_box[0] = pprojp.tile([128, 512], F32,
                                                        tag="ppj", name="pj0")
                                ps_box[1] = pprojp.tile([128, 512], F32,
                                                        tag="ppj", name="pj1")
                            nc.tensor.matmul(
                                ps_box[0][:, :], w_sb[kc][:, ts(2 * mp, 128)],
                                xs3[:, kc, :],
                                start=(kc == 0), stop=(kc == KC - 1))
                            nc.tensor.matmul(
                                ps_box[1][:, :], w_sb[kc][:, ts(2 * mp + 1, 128)],
                                xs3[:, kc, :],
                                start=(kc == 0), stop=(kc == KC - 1))
                        return _step

                    for kc in range(KC):
                        _push(subtag, 2, _mk_step(kc), is_start=(kc == 0))

                    def _epi(mp=mp, n=n, dst=dst, bias_t=bias_t, scale=scale,
                             ps_box=ps_box):
                        for mh in range(2):
                            m = 2 * mp + mh
                            nc.vector.tensor_scalar(
                                dst[m][:, ts(n, 512)], ps_box[mh][:, :],
                                scale, bias_t[:, m : m + 1],
                                ALU.mult, ALU.add)
                    _push(subtag, 0, _epi)

            def push_v(tag, tg, prefetch):
                for tp in range(2):
                    subtag = tag + ("a" if tp == 0 else "b")
                    ps_box = {}

                    def _mk_step(kc, tp=tp, tg=tg, ps_box=ps_box,
                                 prefetch=prefetch):
                        def _step():
                            xs3 = load_x("v", tg)
                            if kc == 0 and tp == 0 and prefetch is not None:
                                load_x(*prefetch)
                            if kc == 0:
                                ps_box[0] = pprojp.tile([128, 512], F32,
                                                        tag="ppj", name="pv0")
                                ps_box[1] = pprojp.tile([128, 512], F32,
                                                        tag="ppj", name="pv1")
                            nc.tensor.matmul(
                                ps_box[0][:, :], xs3[:, kc, ts(2 * tp, 128)],
                                wv_sb[kc][:, :],
                                start=(kc == 0), stop=(kc == KC - 1))
                            nc.tensor.matmul(
                                ps_box[1][:, :], xs3[:, kc, ts(2 * tp + 1, 128)],
                                wv_sb[kc][:, :],
                                start=(kc == 0), stop=(kc == KC - 1))
                        return _step

                    for kc in range(KC):
                        _push(subtag, 2, _mk_step(kc), is_start=(kc == 0))

                    def _epi(tp=tp, tg=tg, ps_box=ps_box):
                        for th in range(2):
                            tt = 4 * tg + 2 * tp + th
                            va3 = v_aug[tt][:, :].rearrange(
                                "p (h x) -> p h x", h=8)
                            ps3 = ps_box[th][:, :].rearrange(
                                "p (h x) -> p h x", h=8)
                            nc.vector.tensor_copy(va3[:, :, 0:64], ps3[:, :, :])
                    _push(subtag, 0, _epi)

            def push_oproj(j):
                # inserted at queue FRONT (groups in order) once norm(j) done
                groups = []
                for jtp in range(4):
                    ps_box = {}
                    steps = []

                    def _mk_step(ei, ec, jtp=jtp, j=j, ps_box=ps_box):
                        def _step():
                            if ei == 0:
                                ps_box[0] = pprojp.tile([128, 512], F32,
                                                        tag="ppj", name="po0")
                                ps_box[1] = pprojp.tile([128, 512], F32,
                                                        tag="ppj", name="po1")
                            nc.tensor.matmul(
                                ps_box[0][:, :], wo_sb[ec][:, ts(2 * jtp, 128)],
                                ctxT[ec][:, ts(j, 512)],
                                start=(ei == 0), stop=(ei == NM - 1))
                            nc.tensor.matmul(
                                ps_box[1][:, :], wo_sb[ec][:, ts(2 * jtp + 1, 128)],
                                ctxT[ec][:, ts(j, 512)],
                                start=(ei == 0), stop=(ei == NM - 1))
                        return _step

                    for ei, ec in enumerate(UORD):
                        steps.append((2, _mk_step(ei, ec), ei == 0))

                    def _epi(jtp=jtp, j=j, ps_box=ps_box):
                        for oh in range(2):
                            st = finp.tile([128, 512], BF, tag="fin", name="st")
                            nc.vector.tensor_copy(st[:, :], ps_box[oh][:, :])
                            nc.sync.dma_start(
                                fT[128 * (2 * jtp + oh) : 128 * (2 * jtp + oh) + 128,
                                   ts(j, 512)],
                                st[:, :])
                    steps.append((0, _epi, False))
                    groups.append(steps)
                for stps in groups:
                    for mms, fn, st in stps:
                        _push(f"o{j}", mms, fn, is_start=st)

            # ---- attention pipeline --------------------------------------
            # step list: (j, u, p, P) — unit-major, block-major
            UORD = (0, 1, 2, 3)
            steps = []
            for j in range(NJ):
                for u in UORD:
                    for p in range(2 * j + 2):
                        steps.append((j, u, p, "u"))
            NSTEP = len(steps)

            unit_state = {}   # (j,u) -> dict(cA, cB)
            step_state = {}   # idx -> dict(sA, sB, eA, eB)
            rs_tiles = {}     # (j, half) -> rs tile
            first_of_block = {j: min(k for k, s in enumerate(steps)
                                     if s[0] == j) for j in range(NJ)}

            def emit_S_half(k, which):
                # which=0: sA (head A, PE rows 0:64); which=1: sB rows 64:128.
                # Split so sB's slot-gate (eB of step k-2) never blocks the
                # filler MMs queued between the halves.
                j, u, p, ph = steps[k]
                if which == 0:
                    mt = "a" if u < 2 else "b"
                    if k == first_of_block[j] or steps[k - 1][1] != u:
                        require(f"q{j}{mt}")
                    if 2 * p + 1 >= 4 * j:
                        require(f"k{j}{mt}")
                i0, i1 = 2 * p, 2 * p + 1
                sX = pscorep.tile([128, 1024], F32, tag="psc",
                                  name="sA" if which == 0 else "sB")
                r0, r1 = (0, 64) if which == 0 else (64, 128)
                for half, i in ((0, i0), (1, i1)):
                    r = i - 4 * j
                    c0 = 128 * r if r > 0 else 0
                    kw = {"tile_position": (64, 0)} if which == 1 else {}
                    nc.tensor.matmul(
                        sX[:, 512 * half + c0 : 512 * half + 512],
                        kT[u][r0:r1, ts(i, 128)],
                        qT[u][r0:r1, 512 * j + c0 : 512 * (j + 1)],
                        start=True, stop=True, **kw)
                st = step_state.setdefault(k, {})
                st["sA" if which == 0 else "sB"] = sX

            def emit_exp_half(k, which):
                j, u, p, ph = steps[k]
                i0, i1 = 2 * p, 2 * p + 1
                st = step_state[k]
                sX = st["sA" if which == 0 else "sB"]
                eX = expp.tile([128, 1024], BF, tag="exp",
                               name="eA" if which == 0 else "eB")
                if i0 - 4 * j == 2:
                    nc.scalar.activation(eX[:, 256:1024], sX[:, 256:1024], AFT.Exp)
                else:
                    nc.scalar.activation(eX[:, :], sX[:, :], AFT.Exp)
                for half, i in ((0, i0), (1, i1)):
                    r = i - 4 * j
                    if r < 0:
                        continue
                    nc.gpsimd.affine_select(
                        out=eX[:, 512 * half + 128 * r : 512 * half + 128 * (r + 1)],
                        in_=eX[:, 512 * half + 128 * r : 512 * half + 128 * (r + 1)],
                        pattern=[[1, 128]],
                        compare_op=ALU.is_ge,
                        fill=zero_fill,
                        base=0,
                        channel_multiplier=-1)
                st["eA" if which == 0 else "eB"] = eX

            def emit_V(k):
                j, u, p, ph = steps[k]
                i0, i1 = 2 * p, 2 * p + 1
                if i1 >= 4 * j:
                    require(f"v{j}a")
                    if i1 >= 4 * j + 2:
                        require(f"v{j}b")
                st = step_state.pop(k)
                eA, eB = st["eA"], st["eB"]
                i_lo, i_hi = 0, 4 * j + 4
                if i0 == i_lo:
                    cA = pctxp.tile([68, 512], F32, tag="pcx", name="cA")
                    cB = pctxp.tile([68, 512], F32, tag="pcx", name="cB")
                    unit_state[(j, u)] = {"cA": cA, "cB": cB}
                us = unit_state[(j, u)]
                cA, cB = us["cA"], us["cB"]
                hA, hB = 2 * u, 2 * u + 1
                for half, i in ((0, i0), (1, i1)):
                    r = i - 4 * j
                    c0 = 128 * r if r > 0 else 0
                    nc.tensor.matmul(
                        cA[:, c0:512], v_aug[i][:, hA * 68 : hA * 68 + 68],
                        eA[:, 512 * half + c0 : 512 * half + 512],
                        start=(i == i_lo), stop=(i == i_hi - 1))
                    nc.tensor.matmul(
                        cB[:, c0:512], v_aug[i][:, hB * 68 : hB * 68 + 68],
                        eB[:, 512 * half + c0 : 512 * half + 512],
                        start=(i == i_lo), stop=(i == i_hi - 1))
                if i1 == i_hi - 1:
                    finish_unit(j, u, add_in=False)

            def accum_rs(j, u, cA, cB, first):
                half = u // 2
                if (j, half) not in rs_tiles:
                    rs_tiles[(j, half)] = rbp.tile(
                        [68, 512], F32, tag="rs", name=f"rs{j}{half}", bufs=3)
                rs = rs_tiles[(j, half)]
                # each unit's rs rides distinct rows of the [64:68) band
                # (zeros elsewhere), so plain adds accumulate the half-pair
                if first:
                    nc.vector.tensor_copy(rs[64:68, :], cA[64:68, :])
                else:
                    nc.vector.tensor_add(rs[64:68, :], rs[64:68, :],
                                         cA[64:68, :])
                nc.vector.tensor_add(rs[64:68, :], rs[64:68, :],
                                     cB[64:68, :])

            def finish_body(j, u):
                us = unit_state.pop((j, u))
                cA, cB = us["cA"], us["cB"]
                accum_rs(j, u, cA, cB, first=(u == UORD[0] or u == UORD[2]))
                nc.vector.tensor_copy(ctxT[u][0:64, ts(j, 512)], cA[0:64, :])
                tm = tmpp.tile([64, 512], BF, tag="tmp", name="tm")
                nc.vector.tensor_copy(tm[:, :], cB[0:64, :])
                nc.sync.dma_start(ctxT[u][64:128, ts(j, 512)], tm[:, :])

            def finish_unit(j, u, add_in):
                us = unit_state.pop((j, u))
                cA, cB = us["cA"], us["cB"]
                if add_in:
                    nc.vector.tensor_add(rs_tiles[(j, u // 2)][64:68, :],
                                         rs_tiles[(j, u // 2)][64:68, :],
                                         cA[64:68, :])
                    nc.vector.tensor_add(rs_tiles[(j, u // 2)][64:68, :],
                                         rs_tiles[(j, u // 2)][64:68, :],
                                         cB[64:68, :])
                    nc.vector.tensor_add(
                        ctxT[u][0:64, ts(j, 512)], ctxT[u][0:64, ts(j, 512)],
                        cA[0:64, :])
                    tm = tmpp.tile([64, 512], BF, tag="tmp", name="tm")
                    nc.vector.tensor_copy(tm[:, :], cB[0:64, :])
                    sh = tmpp.tile([128, 512], BF, tag="tmp2", name="sh",
                                   bufs=2)
                    nc.sync.dma_start(sh[64:128, :], tm[:, :])
                    nc.vector.tensor_add(
                        ctxT[u][64:128, ts(j, 512)], ctxT[u][64:128, ts(j, 512)],
                        sh[64:128, :])
                else:
                    accum_rs(j, u, cA, cB,
                             first=(u == UORD[0] or u == UORD[2]))
                    nc.vector.tensor_copy(ctxT[u][0:64, ts(j, 512)], cA[0:64, :])
                    tm = tmpp.tile([64, 512], BF, tag="tmp", name="tm")
                    nc.vector.tensor_copy(tm[:, :], cB[0:64, :])
                    nc.sync.dma_start(ctxT[u][64:128, ts(j, 512)], tm[:, :])
                half = u // 2
                if u % 2 == 1:
                    norm_half(j, half, u - 1, u)
                if u == UORD[-1]:
                    push_oproj(j)

            def norm_half(j, half, u0, u1):
                drain_to_boundary()
                rs = rs_tiles.pop((j, half))
                # reciprocal_approx_fast only works at partition base 0 —
                # DMA-shift the 4 rs rows down, then approx there (~5x
                # cheaper than the iterative-divide reciprocal and far less
                # Vector-queue blocking).
                rs0 = rbp.tile([4, 512], F32, tag="rs0", name="rs0", bufs=3)
                nc.sync.dma_start(rs0[0:4, :], rs[64:68, :])
                rec = rbp.tile([4, 512], F32, tag="rec", name="rec", bufs=3)
                nc.vector.reciprocal_approx_fast(rec[0:4, :], rs0[0:4, :])
                recb = rbp.tile([4, 512], BF, tag="recb", name="recb", bufs=3)
                nc.vector.tensor_copy(recb[0:4, :], rec[0:4, :])
                for u in (u0, u1):
                    psn = pprojp.tile([128, 512], F32, tag="ppj", name="psn")
                    nc.tensor.matmul(
                        psn[:, :], sel_sb[0:4, ts(u, 128)], recb[0:4, :],
                        start=True, stop=True)
                    nc.vector.tensor_mul(
                        ctxT[u][:, ts(j, 512)], ctxT[u][:, ts(j, 512)],
                        psn[:, :])

            # ---- emission ------------------------------------------------
            # initial x loads for block 0 and prefetch of q1
            load_x("q", 0)
            load_x("k", 0)

            # filler inventory (proj chunks; oproj/norm appended later)
            push_qk("q0", 0, "q", wq_sb, qT, qb_t, 0.125, ("q", 1))
            push_qk("k0", 0, "k", wk_sb, kT, kb_t, 1.0, ("k", 1))
            push_v("v0", 0, ("v", 1))
            push_qk("q1", 1, "q", wq_sb, qT, qb_t, 0.125, ("q", 2))
            push_qk("k1", 1, "k", wk_sb, kT, kb_t, 1.0, ("k", 2))
            push_v("v1", 1, ("v", 2))
            push_qk("q2", 2, "q", wq_sb, qT, qb_t, 0.125, ("q", 3))
            push_qk("k2", 2, "k", wk_sb, kT, kb_t, 1.0, ("k", 3))
            push_v("v2", 2, None)
            push_qk("q3", 3, "q", wq_sb, qT, qb_t, 0.125, None)
            push_qk("k3", 3, "k", wk_sb, kT, kb_t, 1.0, None)
            push_v("v3", 3, None)

            # dense init: q0, k0 (att0 scores/exp need them); v0 rides as
            # filler inside att0 (its first V-MMs arrive a few cycles in).
            require("q0a")
            require("k0a")
            require("v0a")
            for ec in range(NM):
                nc.sync.dma_start(wo_sb[ec][:, :], wo[ts(ec, 128), :])

            # pipeline, one step ahead: per exp-window k the PE queue gets
            # [V(k-1)] [one filler entry] [sA(k+1)+eA] [sB(k+1)+eB] — sized to
            # the window's ~590ns PE slack so exp never starves.
            emit_S_half(0, 0)
            emit_exp_half(0, 0)
            emit_S_half(0, 1)
            emit_exp_half(0, 1)
            for k in range(NSTEP):
                if k >= 1:
                    emit_V(k - 1)
                pump(4)
                if k + 1 < NSTEP:
                    emit_S_half(k + 1, 0)
                    emit_exp_half(k + 1, 0)
                    emit_S_half(k + 1, 1)
                    emit_exp_half(k + 1, 1)
            emit_V(NSTEP - 1)
            # drain remaining fillers (oproj tail); interleave dummy matmuls
            # so HAM stays at 8/8 through the DVE-gated stretches
            while filler_q:
                _run_one()

    nc.compile()
    return nc


def _get_nc():
    global _NC
    if _NC is None:
        _NC = _build()
    return _NC


def build_in_maps(inputs):
    query = np.asarray(inputs["query"], np.float32)
    key = np.asarray(inputs["key"], np.float32)
    value = np.asarray(inputs["value"], np.float32)
    q_w = np.asarray(inputs["q_w"], np.float32)
    q_b = np.asarray(inputs["q_b"], np.float32)
    k_w = np.asarray(inputs["k_w"], np.float32)
    k_b = np.asarray(inputs["k_b"], np.float32)
    v_w = np.asarray(inputs["v_w"], np.float32)
    o_w = np.asarray(inputs["o_w"], np.float32)

    xqT = [np.ascontiguousarray(query[b].T).astype(BF16) for b in range(B)]
    xkT = [np.ascontiguousarray(key[b].T).astype(BF16) for b in range(B)]
    xvT = [np.ascontiguousarray(value[b].T).astype(BF16) for b in range(B)]

    wqT, wkT, wvT, woT, qbt, kbt = [], [], [], [], [], []
    for g in range(2):
        gs = slice(g * G, (g + 1) * G)
        wqT.append(np.ascontiguousarray(q_w[gs, :].T).astype(BF16))
        wkT.append(np.ascontiguousarray(k_w[gs, :].T).astype(BF16))
        wvT.append(np.ascontiguousarray(v_w[gs, :].T).astype(BF16))
        woT.append(np.ascontiguousarray(o_w[:, gs].T).astype(BF16))
        qbt.append(
            np.ascontiguousarray((q_b[gs] / 8.0).reshape(NM, 128).T).astype(
                np.float32
            )
        )
        kbt.append(
            np.ascontiguousarray(k_b[gs].reshape(NM, 128).T).astype(np.float32)
        )

    sel_np = np.zeros((4, G), np.float32)
    for k in range(4):
        for p in range(G):
            hp, pp = p // 128, p % 128
            if k == (2 * hp + (pp // 64)) % 4:
                sel_np[k, p] = 1.0
    sel_np = sel_np.astype(BF16)

    in_maps = []
    for b in range(B):
        for g in range(2):
            in_maps.append(
                {
                    "xq": xqT[b],
                    "xk": xkT[b],
                    "xv": xvT[b],
                    "wq": wqT[g],
                    "wk": wkT[g],
                    "wv": wvT[g],
                    "wo": woT[g],
                    "qb": qbt[g],
                    "kb": kbt[g],
                    "sel": sel_np,
                }
            )

    return in_maps


def kernel(**inputs):
    nc = _get_nc()
    in_maps = build_in_maps(inputs)
    res = bass_utils.run_bass_kernel_spmd(nc, in_maps, core_ids=list(range(8)))

    o_b = np.asarray(inputs["o_b"], np.float32)
    v_b = np.asarray(inputs["v_b"], np.float32)
    o_w = np.asarray(inputs["o_w"], np.float32)
    corr = (o_b + v_b @ o_w.T).astype(np.float32)  # softmax rows sum to 1
    out = np.empty((B, S, E), np.float32)
    for b in range(B):
        acc = (res.results[2 * b]["fT"].astype(np.float32)
               + res.results[2 * b + 1]["fT"].astype(np.float32))
        out[b] = acc.T + corr[None, :]
    return out


# revision 4
# speedup vs baseline: 1.0301x; 1.0301x over previous
"""Multi-head attention (B=4, S=2048, E=1024, H=16, causal) on 8 Trainium2 cores.

Sharding: core = (batch b, head-group g) — 4 batches x 2 groups of 8 heads.

v4: single fine-grained software pipeline.  The attention stream is a flat
list of ip-steps (unit-major, block-major); scores are emitted ONE step
ahead of the exp window (queue order per cycle: V(k-1), S(k+1), fillers)
so ScalarE exp runs back-to-back while the PE executes attn@V of the
previous step, scores of the next, plus projection/output-projection
matmul chunks pulled from a filler queue.  kv(j)/q(j) availability is
enforced by require() seams that drain the filler queue densely at the
first step that needs them.  Body+diag of each unit share one PSUM
accumulation (no ctx re-adds); row-sums ride the ones-columns of v_aug
and are normalized per half-pair with reciprocal_approx_fast.
DMA triggers never ride the Scalar queue (ACT is the pacer).
"""

import os
import sys

for _p in ("/opt/trn_rl_repo", "/root/.axon_site/_ro/trn_rl_repo"):
    if os.path.isdir(_p) and _p not in sys.path:
        sys.path.append(_p)

from collections import deque

import numpy as np
import ml_dtypes

import concourse.bacc as bacc
import concourse.mybir as mybir
from concourse import tile
from concourse import bass_utils
from concourse.bass import ts

BF16 = ml_dtypes.bfloat16
F32 = mybir.dt.float32
BF = mybir.dt.bfloat16
AFT = mybir.ActivationFunctionType
ALU = mybir.AluOpType

B, S, E = 4, 2048, 1024
H, D = 16, 64
G = 512            # head dims per core (8 heads)
KC = E // 128      # contraction chunks for projections
NM = G // 128      # m-tiles of the group dim
NJ = S // 512      # 512-wide token column blocks
NT = S // 128      # 128-wide token tiles

_NC = None


def _build():
    nc = bacc.Bacc("TRN2", target_bir_lowering=False, debug=False, num_devices=8)

    xq = nc.dram_tensor("xq", (E, S), BF, kind="ExternalInput").ap()
    xk = nc.dram_tensor("xk", (E, S), BF, kind="ExternalInput").ap()
    xv = nc.dram_tensor("xv", (E, S), BF, kind="ExternalInput").ap()
    wq = nc.dram_tensor("wq", (E, G), BF, kind="ExternalInput").ap()
    wk = nc.dram_tensor("wk", (E, G), BF, kind="ExternalInput").ap()
    wv = nc.dram_tensor("wv", (E, G), BF, kind="ExternalInput").ap()
    wo = nc.dram_tensor("wo", (G, E), BF, kind="ExternalInput").ap()
    qb = nc.dram_tensor("qb", (128, NM), F32, kind="ExternalInput").ap()
    kb = nc.dram_tensor("kb", (128, NM), F32, kind="ExternalInput").ap()
    sel = nc.dram_tensor("sel", (4, G), BF, kind="ExternalInput").ap()
    fT = nc.dram_tensor("fT", (E, S), BF, kind="ExternalOutput").ap()

    with tile.TileContext(nc) as tc:
        with (
            tc.tile_pool(name="cst", bufs=2) as cst,
            tc.tile_pool(name="wsb", bufs=24) as wsb,
            tc.tile_pool(name="xs", bufs=6) as xsp,
            tc.tile_pool(name="qt", bufs=8) as qtp,
            tc.tile_pool(name="va", bufs=16) as vap,
            tc.tile_pool(name="ctx", bufs=4) as ctxp,
            tc.tile_pool(name="exp", bufs=10) as expp,
            tc.tile_pool(name="wo", bufs=4) as wop,
            tc.tile_pool(name="fin", bufs=4) as finp,
            tc.tile_pool(name="rb", bufs=9) as rbp,
            tc.tile_pool(name="tmp", bufs=4) as tmpp,
            tc.tile_pool(name="psc", bufs=2, space="PSUM") as pscorep,
            tc.tile_pool(name="pcx", bufs=2, space="PSUM") as pctxp,
            tc.tile_pool(name="ppj", bufs=2, space="PSUM") as pprojp,
        ):
            qb_t = cst.tile([128, NM], F32, tag="cst")
            kb_t = cst.tile([128, NM], F32, tag="cst")
            sel_sb = cst.tile([4, G], BF, tag="sel", name="sel_sb")

            zero_fill = nc.gpsimd.to_reg(0.0)

            # Warm the ScalarE Exp table long before the first real exp.
            warm = cst.tile([1, 8], F32, tag="warm", name="warm")
            nc.vector.memset(warm[:, :], 0.0)
            nc.scalar.activation(warm[:, :], warm[:, :], AFT.Exp)

            # PE HAM warm-up on a zeroed tile while the initial DMAs land.
            wmt = cst.tile([128, 384], BF, tag="wmt", name="wmt")
            nc.vector.memset(wmt[:, :], 0.0)
            for wi in range(16):
                wps = pprojp.tile([128, 512], F32, tag="ppj", name=f"wps{wi}")
                nc.tensor.matmul(
                    wps[:, 0:256], wmt[:, 0:128], wmt[:, 128:384],
                    start=True, stop=True)
            nc.scalar.activation(warm[:, :], warm[:, :], AFT.Exp)

            # constant ones-slots pattern for v_aug cols [64..68) per head
            ones_c = cst.tile([128, 8 * 68], BF, tag="ones", name="ones_c")
            ones_c3 = ones_c[:, :].rearrange("p (h x) -> p h x", h=8)
            nc.vector.memset(ones_c3[:, :, 64:68], 0.0)
            for h in range(8):
                nc.vector.memset(
                    ones_c3[:, h : h + 1, 64 + (h % 4) : 65 + (h % 4)], 1.0)

            qT = [qtp.tile([128, S], BF, tag="qt", name=f"qT{m}") for m in range(NM)]
            kT = [qtp.tile([128, S], BF, tag="qt", name=f"kT{m}") for m in range(NM)]
            ctxT = [ctxp.tile([128, S], BF, tag="ctx", name=f"ctxT{m}")
                    for m in range(NM)]
            v_aug = [vap.tile([128, 8 * 68], BF, tag="va", name=f"va{t}")
                     for t in range(NT)]

            nc.gpsimd.dma_start(qb_t[:, :], qb[:, :])
            nc.gpsimd.dma_start(kb_t[:, :], kb[:, :])
            nc.gpsimd.dma_start(sel_sb[0:4, :], sel[:, :])

            # q/k/v weights resident for the whole kernel
            wq_sb = [wsb.tile([128, G], BF, tag="w", name=f"wq{kc}") for kc in range(KC)]
            wk_sb = [wsb.tile([128, G], BF, tag="w", name=f"wk{kc}") for kc in range(KC)]
            wv_sb = [wsb.tile([128, G], BF, tag="w", name=f"wv{kc}") for kc in range(KC)]
            for kc in range(KC):
                nc.gpsimd.dma_start(wq_sb[kc][:, :], wq[ts(kc, 128), :])
            for kc in range(KC):
                nc.sync.dma_start(wk_sb[kc][:, :], wk[ts(kc, 128), :])
            for kc in range(KC):
                nc.gpsimd.dma_start(wv_sb[kc][:, :], wv[ts(kc, 128), :])
            wo_sb = [wop.tile([128, E], BF, tag="wo", name=f"wo{ec}") for ec in range(NM)]
            for t in range(NT):
                va3i = v_aug[t][:, :].rearrange("p (h x) -> p h x", h=8)
                nc.gpsimd.tensor_copy(va3i[:, :, 64:68], ones_c3[:, :, 64:68])

            # ---- x-block staging (fine first block, 2 triggers later) ----
            _x_ap = {"q": xq, "k": xk, "v": xv}
            _xsb = {}

            def load_x(which, n):
                key = (which, n)
                if key in _xsb:
                    return _xsb[key]
                xsb = xsp.tile([128, KC * 512], BF, tag="xs", name=f"x{which}{n}")
                xs3 = xsb[:, :].rearrange("p (k c) -> p k c", k=KC)
                ap = _x_ap[which]
                if n == 0:
                    for kc in range(KC):
                        nc.sync.dma_start(xs3[:, kc, :], ap[ts(kc, 128), ts(n, 512)])
                else:
                    src = ap[:, ts(n, 512)].rearrange("(k p) c -> p k c", p=128)
                    nc.sync.dma_start(xs3[:, 0:4, :], src[:, 0:4, :])
                    nc.sync.dma_start(xs3[:, 4:8, :], src[:, 4:8, :])
                _xsb[key] = xs3
                return xs3

            # ---- filler queue --------------------------------------------
            # entries: (tag, n_mms, fn).  pump(b) emits entries until b MM
            # slots are consumed; require(tag) drains densely until every
            # entry of `tag` has been emitted.
            filler_q = deque()
            _remaining = {}
            done_tags = set()

            def _push(tag, mms, fn, is_start=False, front=False):
                _remaining[tag] = _remaining.get(tag, 0) + 1
                if front:
                    filler_q.appendleft((tag, mms, fn, is_start))
                else:
                    filler_q.append((tag, mms, fn, is_start))

            def _run_one():
                tag, mms, fn, _st = filler_q.popleft()
                fn()
                _remaining[tag] -= 1
                if _remaining[tag] == 0:
                    done_tags.add(tag)
                return max(mms, 1)

            _dummy_n = [0]

            def pump(budget):
                while budget > 0 and filler_q:
                    budget -= _run_one()
                if budget >= 2:
                    # queue dry: keep PE duty high so the HAM clock gate
                    # stays at 8/8 (dummies are ~107ns N=128 matmuls)
                    _dummy_n[0] += 1
                    wp = pprojp.tile([128, 512], F32, tag="ppj",
                                     name=f"dum{_dummy_n[0]}")
                    nc.tensor.matmul(wp[:, 0:128], wmt[:, 0:128],
                                     wmt[:, 128:256], start=True, stop=True)

            def require(tag):
                if tag in done_tags or tag not in _remaining:
                    return
                while tag not in done_tags:
                    _run_one()

            def drain_to_boundary():
                # Flush the remainder of any half-emitted chunk so that no
                # inline PSUM allocation (psn) can slot-wait on an epilogue
                # that would otherwise be emitted BEHIND it (PE-queue cycle).
                while filler_q and not filler_q[0][3]:
                    _run_one()

            # ---- projection chunk generators -----------------------------
            def push_qk(tag, n, which, w_sb, dst, bias_t, scale, prefetch):
                # one chain (one PSUM tile) per 128-row m-tile: entries are
                # single matmuls, so pump pacing is fine-grained and a chain
                # never holds more than one ppj slot before its epilogue
                for m in range(NM):
                    subtag = f"{tag}m{m}"
                    ps_box = {}

                    def _mk_step(kc, m=m, n=n, which=which, w_sb=w_sb,
                                 ps_box=ps_box, prefetch=prefetch):
                        def _step():
                            xs3 = load_x(which, n)
                            if kc == 0 and m == 0 and prefetch is not None:
                                load_x(*prefetch)
                            if kc == 0:
                                ps_box[0] = pprojp.tile([128, 512], F32,
                                                        tag="ppj", name="pj0")
                            nc.tensor.matmul(
                                ps_box[0][:, :], w_sb[kc][:, ts(m, 128)],
                                xs3[:, kc, :],
                                start=(kc == 0), stop=(kc == KC - 1))
                        return _step

                    for kc in range(KC):
                        _push(subtag, 1, _mk_step(kc), is_start=(kc == 0))

                    def _epi(m=m, n=n, dst=dst, bias_t=bias_t, scale=scale,
                             ps_box=ps_box):
                        nc.vector.tensor_scalar(
                            dst[m][:, ts(n, 512)], ps_box[0][:, :],
                            scale, bias_t[:, m : m + 1],
                            ALU.mult, ALU.add)
                    _push(subtag, 0, _epi)

            def push_v(tag, tg, prefetch):
                for th in range(4):
                    subtag = tag + ("a" if th < 2 else "b")
                    ps_box = {}

                    def _mk_step(kc, th=th, tg=tg, ps_box=ps_box,
                                 prefetch=prefetch):
                        def _step():
                            xs3 = load_x("v", tg)
                            if kc == 0 and th == 0 and prefetch is not None:
                                load_x(*prefetch)
                            if kc == 0:
                                ps_box[0] = pprojp.tile([128, 512], F32,
                                                        tag="ppj", name="pv0")
                            nc.tensor.matmul(
                                ps_box[0][:, :], xs3[:, kc, ts(th, 128)],
                                wv_sb[kc][:, :],
                                start=(kc == 0), stop=(kc == KC - 1))
                        return _step

                    for kc in range(KC):
                        _push(subtag, 1, _mk_step(kc), is_start=(kc == 0))

                    def _epi(th=th, tg=tg, ps_box=ps_box):
                        tt = 4 * tg + th
                        va3 = v_aug[tt][:, :].rearrange("p (h x) -> p h x", h=8)
                        ps3 = ps_box[0][:, :].rearrange("p (h x) -> p h x", h=8)
                        nc.vector.tensor_copy(va3[:, :, 0:64], ps3[:, :, :])
                    _push(subtag, 0, _epi)

            def push_oproj(j):
                # inserted at queue FRONT (groups in order) once norm(j) done
                groups = []
                for jtp in range(4):
                    ps_box = {}
                    steps = []

                    def _mk_step(ei, ec, jtp=jtp, j=j, ps_box=ps_box):
                        def _step():
                            if ei == 0:
                                ps_box[0] = pprojp.tile([128, 512], F32,
                                                        tag="ppj", name="po0")
                                ps_box[1] = pprojp.tile([128, 512], F32,
                                                        tag="ppj", name="po1")
                            nc.tensor.matmul(
                                ps_box[0][:, :], wo_sb[ec][:, ts(2 * jtp, 128)],
                                ctxT[ec][:, ts(j, 512)],
                                start=(ei == 0), stop=(ei == NM - 1))
                            nc.tensor.matmul(
                                ps_box[1][:, :], wo_sb[ec][:, ts(2 * jtp + 1, 128)],
                                ctxT[ec][:, ts(j, 512)],
                                start=(ei == 0), stop=(ei == NM - 1))
                        return _step

                    for ei, ec in enumerate(UORD):
                        steps.append((2, _mk_step(ei, ec), ei == 0))

                    def _epi(jtp=jtp, j=j, ps_box=ps_box):
                        for oh in range(2):
                            st = finp.tile([128, 512], BF, tag="fin", name="st")
                            nc.vector.tensor_copy(st[:, :], ps_box[oh][:, :])
                            nc.sync.dma_start(
                                fT[128 * (2 * jtp + oh) : 128 * (2 * jtp + oh) + 128,
                                   ts(j, 512)],
                                st[:, :])
                    steps.append((0, _epi, False))
                    groups.append(steps)
                for stps in groups:
                    for mms, fn, st in stps:
                        _push(f"o{j}", mms, fn, is_start=st)

            # ---- attention pipeline --------------------------------------
            # step list: (j, u, p, P) — unit-major, block-major
            UORD = (0, 1, 2, 3)
            steps = []
            for j in range(NJ):
                for u in UORD:
                    for p in range(2 * j + 2):
                        steps.append((j, u, p, "u"))
            NSTEP = len(steps)

            unit_state = {}   # (j,u) -> dict(cA, cB)
            step_state = {}   # idx -> dict(sA, sB, eA, eB)
            rs_tiles = {}     # (j, half) -> rs tile
            first_of_block = {j: min(k for k, s in enumerate(steps)
                                     if s[0] == j) for j in range(NJ)}

            def emit_S_half(k, which):
                # which=0: sA (head A, PE rows 0:64); which=1: sB rows 64:128.
                # Split so sB's slot-gate (eB of step k-2) never blocks the
                # filler MMs queued between the halves.
                j, u, p, ph = steps[k]
                if which == 0:
                    if k == first_of_block[j] or steps[k - 1][1] != u:
                        require(f"q{j}m{u}")
                    if 2 * p + 1 >= 4 * j:
                        require(f"k{j}m{u}")
                i0, i1 = 2 * p, 2 * p + 1
                sX = pscorep.tile([128, 1024], F32, tag="psc",
                                  name="sA" if which == 0 else "sB")
                r0, r1 = (0, 64) if which == 0 else (64, 128)
                for half, i in ((0, i0), (1, i1)):
                    r = i - 4 * j
                    c0 = 128 * r if r > 0 else 0
                    kw = {"tile_position": (64, 0)} if which == 1 else {}
                    nc.tensor.matmul(
                        sX[:, 512 * half + c0 : 512 * half + 512],
                        kT[u][r0:r1, ts(i, 128)],
                        qT[u][r0:r1, 512 * j + c0 : 512 * (j + 1)],
                        start=True, stop=True, **kw)
                st = step_state.setdefault(k, {})
                st["sA" if which == 0 else "sB"] = sX

            def emit_exp_half(k, which):
                j, u, p, ph = steps[k]
                i0, i1 = 2 * p, 2 * p + 1
                st = step_state[k]
                sX = st["sA" if which == 0 else "sB"]
                eX = expp.tile([128, 1024], BF, tag="exp",
                               name="eA" if which == 0 else "eB")
                if i0 - 4 * j == 2:
                    nc.scalar.activation(eX[:, 256:1024], sX[:, 256:1024], AFT.Exp)
                else:
                    nc.scalar.activation(eX[:, :], sX[:, :], AFT.Exp)
                for half, i in ((0, i0), (1, i1)):
                    r = i - 4 * j
                    if r < 0:
                        continue
                    nc.gpsimd.affine_select(
                        out=eX[:, 512 * half + 128 * r : 512 * half + 128 * (r + 1)],
                        in_=eX[:, 512 * half + 128 * r : 512 * half + 128 * (r + 1)],
                        pattern=[[1, 128]],
                        compare_op=ALU.is_ge,
                        fill=zero_fill,
                        base=0,
                        channel_multiplier=-1)
                st["eA" if which == 0 else "eB"] = eX

            def emit_V(k):
                j, u, p, ph = steps[k]
                i0, i1 = 2 * p, 2 * p + 1
                if i1 >= 4 * j:
                    require(f"v{j}a")
                    if i1 >= 4 * j + 2:
                        require(f"v{j}b")
                st = step_state.pop(k)
                eA, eB = st["eA"], st["eB"]
                i_lo, i_hi = 0, 4 * j + 4
                if i0 == i_lo:
                    cA = pctxp.tile([68, 512], F32, tag="pcx", name="cA")
                    cB = pctxp.tile([68, 512], F32, tag="pcx", name="cB")
                    unit_state[(j, u)] = {"cA": cA, "cB": cB}
                us = unit_state[(j, u)]
                cA, cB = us["cA"], us["cB"]
                hA, hB = 2 * u, 2 * u + 1
                for half, i in ((0, i0), (1, i1)):
                    r = i - 4 * j
                    c0 = 128 * r if r > 0 else 0
                    nc.tensor.matmul(
                        cA[:, c0:512], v_aug[i][:, hA * 68 : hA * 68 + 68],
                        eA[:, 512 * half + c0 : 512 * half + 512],
                        start=(i == i_lo), stop=(i == i_hi - 1))
                    nc.tensor.matmul(
                        cB[:, c0:512], v_aug[i][:, hB * 68 : hB * 68 + 68],
                        eB[:, 512 * half + c0 : 512 * half + 512],
                        start=(i == i_lo), stop=(i == i_hi - 1))
                if i1 == i_hi - 1:
                    finish_unit(j, u, add_in=False)

            def accum_rs(j, u, cA, cB, first):
                half = u // 2
                if (j, half) not in rs_tiles:
                    rs_tiles[(j, half)] = rbp.tile(
                        [68, 512], F32, tag="rs", name=f"rs{j}{half}", bufs=3)
                rs = rs_tiles[(j, half)]
                # each unit's rs rides distinct rows of the [64:68) band
                # (zeros elsewhere), so plain adds accumulate the half-pair
                if first:
                    nc.vector.tensor_copy(rs[64:68, :], cA[64:68, :])
                else:
                    nc.vector.tensor_add(rs[64:68, :], rs[64:68, :],
                                         cA[64:68, :])
                nc.vector.tensor_add(rs[64:68, :], rs[64:68, :],
                                     cB[64:68, :])

            def finish_body(j, u):
                us = unit_state.pop((j, u))
                cA, cB = us["cA"], us["cB"]
                accum_rs(j, u, cA, cB, first=(u == UORD[0] or u == UORD[2]))
                nc.vector.tensor_copy(ctxT[u][0:64, ts(j, 512)], cA[0:64, :])
                tm = tmpp.tile([64, 512], BF, tag="tmp", name="tm")
                nc.vector.tensor_copy(tm[:, :], cB[0:64, :])
                nc.sync.dma_start(ctxT[u][64:128, ts(j, 512)], tm[:, :])

            def finish_unit(j, u, add_in):
                us = unit_state.pop((j, u))
                cA, cB = us["cA"], us["cB"]
                if add_in:
                    nc.vector.tensor_add(rs_tiles[(j, u // 2)][64:68, :],
                                         rs_tiles[(j, u // 2)][64:68, :],
                                         cA[64:68, :])
                    nc.vector.tensor_add(rs_tiles[(j, u // 2)][64:68, :],
                                         rs_tiles[(j, u // 2)][64:68, :],
                                         cB[64:68, :])
                    nc.vector.tensor_add(
                        ctxT[u][0:64, ts(j, 512)], ctxT[u][0:64, ts(j, 512)],
                        cA[0:64, :])
                    tm = tmpp.tile([64, 512], BF, tag="tmp", name="tm")
                    nc.vector.tensor_copy(tm[:, :], cB[0:64, :])
                    sh = tmpp.tile([128, 512], BF, tag="tmp2", name="sh",
                                   bufs=2)
                    nc.sync.dma_start(sh[64:128, :], tm[:, :])
                    nc.vector.tensor_add(
                        ctxT[u][64:128, ts(j, 512)], ctxT[u][64:128, ts(j, 512)],
                        sh[64:128, :])
                else:
                    accum_rs(j, u, cA, cB,
                             first=(u == UORD[0] or u == UORD[2]))
                    nc.vector.tensor_copy(ctxT[u][0:64, ts(j, 512)], cA[0:64, :])
                    tm = tmpp.tile([64, 512], BF, tag="tmp", name="tm")
                    nc.vector.tensor_copy(tm[:, :], cB[0:64, :])
                    nc.sync.dma_start(ctxT[u][64:128, ts(j, 512)], tm[:, :])
                half = u // 2
                if u % 2 == 1:
                    norm_half(j, half, u - 1, u)
                if u == UORD[-1]:
                    push_oproj(j)

            def norm_half(j, half, u0, u1):
                drain_to_boundary()
                rs = rs_tiles.pop((j, half))
                # reciprocal_approx_fast only works at partition base 0 —
                # DMA-shift the 4 rs rows down, then approx there (~5x
                # cheaper than the iterative-divide reciprocal and far less
                # Vector-queue blocking).
                rs0 = rbp.tile([4, 512], F32, tag="rs0", name="rs0", bufs=3)
                nc.sync.dma_start(rs0[0:4, :], rs[64:68, :])
                rec = rbp.tile([4, 512], F32, tag="rec", name="rec", bufs=3)
                nc.vector.reciprocal_approx_fast(rec[0:4, :], rs0[0:4, :])
                recb = rbp.tile([4, 512], BF, tag="recb", name="recb", bufs=3)
                nc.vector.tensor_copy(recb[0:4, :], rec[0:4, :])
                for u in (u0, u1):
                    psn = pprojp.tile([128, 512], F32, tag="ppj", name="psn")
                    nc.tensor.matmul(
                        psn[:, :], sel_sb[0:4, ts(u, 128)], recb[0:4, :],
                        start=True, stop=True)
                    nc.vector.tensor_mul(
                        ctxT[u][:, ts(j, 512)], ctxT[u][:, ts(j, 512)],
                        psn[:, :])

            # ---- emission ------------------------------------------------
            # initial x loads for block 0 and prefetch of q1
            load_x("q", 0)
            load_x("k", 0)

            # filler inventory (proj chunks; oproj/norm appended later)
            push_qk("q0", 0, "q", wq_sb, qT, qb_t, 0.125, ("q", 1))
            push_qk("k0", 0, "k", wk_sb, kT, kb_t, 1.0, ("k", 1))
            push_v("v0", 0, ("v", 1))
            push_qk("q1", 1, "q", wq_sb, qT, qb_t, 0.125, ("q", 2))
            push_qk("k1", 1, "k", wk_sb, kT, kb_t, 1.0, ("k", 2))
            push_v("v1", 1, ("v", 2))
            push_qk("q2", 2, "q", wq_sb, qT, qb_t, 0.125, ("q", 3))
            push_qk("k2", 2, "k", wk_sb, kT, kb_t, 1.0, ("k", 3))
            push_v("v2", 2, None)
            push_qk("q3", 3, "q", wq_sb, qT, qb_t, 0.125, None)
            push_qk("k3", 3, "k", wk_sb, kT, kb_t, 1.0, None)
            push_v("v3", 3, None)

            # dense init: q0, k0 (att0 scores/exp need them); v0 rides as
            # filler inside att0 (its first V-MMs arrive a few cycles in).
            require("q0m0")
            require("k0m0")
            require("v0a")
            for ec in range(NM):
                nc.sync.dma_start(wo_sb[ec][:, :], wo[ts(ec, 128), :])

            # pipeline, one step ahead: per exp-window k the PE queue gets
            # [V(k-1)] [one filler entry] [sA(k+1)+eA] [sB(k+1)+eB] — sized to
            # the window's ~590ns PE slack so exp never starves.
            emit_S_half(0, 0)
            emit_exp_half(0, 0)
            emit_S_half(0, 1)
            emit_exp_half(0, 1)
            for k in range(NSTEP):
                if k >= 1:
                    emit_V(k - 1)
                pump(4)
                if k + 1 < NSTEP:
                    emit_S_half(k + 1, 0)
                    emit_exp_half(k + 1, 0)
                    emit_S_half(k + 1, 1)
                    emit_exp_half(k + 1, 1)
            emit_V(NSTEP - 1)
            # drain remaining fillers (oproj tail); interleave dummy matmuls
            # so HAM stays at 8/8 through the DVE-gated stretches
            while filler_q:
                _run_one()

    nc.compile()
    return nc


def _get_nc():
    global _NC
    if _NC is None:
        _NC = _build()
    return _NC


def build_in_maps(inputs):
    query = np.asarray(inputs["query"], np.float32)
    key = np.asarray(inputs["key"], np.float32)
    value = np.asarray(inputs["value"], np.float32)
    q_w = np.asarray(inputs["q_w"], np.float32)
    q_b = np.asarray(inputs["q_b"], np.float32)
    k_w = np.asarray(inputs["k_w"], np.float32)
    k_b = np.asarray(inputs["k_b"], np.float32)
    v_w = np.asarray(inputs["v_w"], np.float32)
    o_w = np.asarray(inputs["o_w"], np.float32)

    xqT = [np.ascontiguousarray(query[b].T).astype(BF16) for b in range(B)]
    xkT = [np.ascontiguousarray(key[b].T).astype(BF16) for b in range(B)]
    xvT = [np.ascontiguousarray(value[b].T).astype(BF16) for b in range(B)]

    wqT, wkT, wvT, woT, qbt, kbt = [], [], [], [], [], []
    for g in range(2):
        gs = slice(g * G, (g + 1) * G)
        wqT.append(np.ascontiguousarray(q_w[gs, :].T).astype(BF16))
        wkT.append(np.ascontiguousarray(k_w[gs, :].T).astype(BF16))
        wvT.append(np.ascontiguousarray(v_w[gs, :].T).astype(BF16))
        woT.append(np.ascontiguousarray(o_w[:, gs].T).astype(BF16))
        qbt.append(
            np.ascontiguousarray((q_b[gs] / 8.0).reshape(NM, 128).T).astype(
                np.float32
            )
        )
        kbt.append(
            np.ascontiguousarray(k_b[gs].reshape(NM, 128).T).astype(np.float32)
        )

    sel_np = np.zeros((4, G), np.float32)
    for k in range(4):
        for p in range(G):
            hp, pp = p // 128, p % 128
            if k == (2 * hp + (pp // 64)) % 4:
                sel_np[k, p] = 1.0
    sel_np = sel_np.astype(BF16)

    in_maps = []
    for b in range(B):
        for g in range(2):
            in_maps.append(
                {
                    "xq": xqT[b],
                    "xk": xkT[b],
                    "xv": xvT[b],
                    "wq": wqT[g],
                    "wk": wkT[g],
                    "wv": wvT[g],
                    "wo": woT[g],
                    "qb": qbt[g],
                    "kb": kbt[g],
                    "sel": sel_np,
                }
            )

    return in_maps


def kernel(**inputs):
    nc = _get_nc()
    in_maps = build_in_maps(inputs)
    res = bass_utils.run_bass_kernel_spmd(nc, in_maps, core_ids=list(range(8)))

    o_b = np.asarray(inputs["o_b"], np.float32)
    v_b = np.asarray(inputs["v_b"], np.float32)
    o_w = np.asarray(inputs["o_w"], np.float32)
    corr = (o_b + v_b @ o_w.T).astype(np.float32)  # softmax rows sum to 1
    out = np.empty((B, S, E), np.float32)
    for b in range(B):
        acc = (res.results[2 * b]["fT"].astype(np.float32)
               + res.results[2 * b + 1]["fT"].astype(np.float32))
        out[b] = acc.T + corr[None, :]
    return out
